# revision 11
# baseline (speedup 1.0000x reference)
"""Trainium2 Bass kernel for nn_Attend (l2-distance attention with zero-kv).

Reference computation (per b,h):
    k' = [0; k], v' = [0; v]                       (prepend zero kv)
    scores[i,j] = (2 q_i.k'_j - |q_i|^2 - |k'_j|^2) * (D+2)^-0.5
    causal: j <= i+1 in padded index space
    out = softmax(scores) @ v'

Kernel algebra: softmax is invariant to the per-row constant -scale*|q_i|^2,
so with p~[i,j] = exp(2*scale*q_i.k_j) * exp(-scale*|k_j|^2) and the zero
column contributing exp(0)=1 to the denominator only:
    out_i = (sum_j p~ * v_j) / (1 + sum_j p~)

Layout: scores are computed TRANSPOSED ([kv, q]) so that P^T is directly the
moving operand of the PV matmul (no P transposes).  The exp(-scale*|k|^2)
factor is folded into the PV stationary operand [V | 1] per kv partition.

Sharding: 32 (b,h) pairs -> 4 heads per core, 8 cores, pure data parallel.
"""

import sys

for _p in ("/opt/trn_rl_repo", "/root/.axon_site"):
    if _p not in sys.path:
        sys.path.insert(0, _p)

import numpy as np

B, H, N, D = 2, 16, 2048, 64
NCORES = 8
HPC = (B * H) // NCORES          # heads per core = 4
SCALE = float((D + 2) ** -0.5)   # augmented head dim, matches reference
NB = N // 128                    # kv blocks of 128 = 16
NQT = N // 512                   # q tiles of 512 = 4

_BUILT = {}


def _build(qk_dt="float32r", pv_dt="float32r", hpc=HPC, n=N, debug_taps=False, stage=99):
    """Build + finalize the SPMD Bass program (one core's view)."""
    NB = n // 128
    NQT = n // 512
    import concourse.mybir as mybir
    import concourse.tile as tile
    from concourse import bacc
    from concourse.masks import make_identity

    f32 = mybir.dt.float32
    QKD = getattr(mybir.dt, qk_dt)
    PVD = getattr(mybir.dt, pv_dt)
    Exp = mybir.ActivationFunctionType.Exp
    mult = mybir.AluOpType.mult
    add = mybir.AluOpType.add

    nc = bacc.Bacc("TRN2", target_bir_lowering=False, debug=False)
    q_p = nc.declare_dram_parameter("q", [hpc, n, D], f32, isOutput=False)
    k_p = nc.declare_dram_parameter("k", [hpc, n, D], f32, isOutput=False)
    v_p = nc.declare_dram_parameter("v", [hpc, n, D], f32, isOutput=False)
    m_p = nc.declare_dram_parameter("masks", [128, 4 * 512], f32, isOutput=False)
    o_p = nc.declare_dram_parameter("out", [hpc, n, D], f32, isOutput=True)
    if debug_taps:
        d_qT = nc.declare_dram_parameter("d_qT", [64, n], f32, isOutput=True)
        d_kT = nc.declare_dram_parameter("d_kT", [64, n], f32, isOutput=True)
        d_vo = nc.declare_dram_parameter("d_vo", [128, NB, 65], f32, isOutput=True)
        d_pt = nc.declare_dram_parameter("d_pt", [128, 1024], f32, isOutput=True)
        d_acc = nc.declare_dram_parameter("d_acc", [65, 512], f32, isOutput=True)

    with tile.TileContext(nc) as tc:
        with (
            tc.tile_pool(name="const", bufs=1) as constp,
            tc.tile_pool(name="io", bufs=2) as iop,
            tc.tile_pool(name="kqt", bufs=2) as kqtp,
            tc.tile_pool(name="pt", bufs=3) as ptp,
            tc.tile_pool(name="fin", bufs=2) as finp,
            tc.tile_pool(name="ps_s", bufs=2, space="PSUM") as ps_s,
            tc.tile_pool(name="ps_a", bufs=2, space="PSUM") as ps_a,
            tc.tile_pool(name="ps_t", bufs=2, space="PSUM") as ps_t,
        ):
            ident = constp.tile([128, 128], f32, tag="ident")
            make_identity(nc, ident[:])
            maskt = constp.tile([128, 4 * 512], f32, tag="maskt")
            nc.sync.dma_start(out=maskt[:], in_=m_p[:])

            for h in range(hpc):
                # ---- load head inputs --------------------------------
                qn = iop.tile([128, NB, 64], f32, tag="qn")
                kn = iop.tile([128, NB, 64], f32, tag="kn")
                vn = iop.tile([128, NB, 64], f32, tag="vn")
                vo = iop.tile([128, NB, 65], PVD, tag="vo")
                nc.sync.dma_start(
                    out=qn[:], in_=q_p[h].rearrange("(b p) d -> p b d", p=128)
                )
                nc.sync.dma_start(
                    out=kn[:], in_=k_p[h].rearrange("(b p) d -> p b d", p=128)
                )
                nc.sync.dma_start(
                    out=vn[:], in_=v_p[h].rearrange("(b p) d -> p b d", p=128)
                )

                if stage < 2:
                    ot0 = finp.tile([128, 64], f32, tag="ot0")
                    nc.vector.tensor_copy(ot0[:], qn[:, 0, :])
                    nc.sync.dma_start(out=o_p[h, 0:128, :], in_=ot0[:])
                    continue
                # ---- ek = exp(-scale*|k|^2) folded into [V|1] --------
                ksqs = iop.tile([128, NB], f32, tag="ksqs")
                scr = iop.tile([128, 64], f32, tag="scr")
                for b in range(NB):
                    nc.vector.tensor_mul(scr[:], kn[:, b, :], kn[:, b, :])
                    nc.vector.tensor_reduce(
                        ksqs[:, b : b + 1], scr[:], mybir.AxisListType.X, add
                    )
                ek = iop.tile([128, NB], f32, tag="ek")
                nc.scalar.activation(ek[:], ksqs[:], Exp, scale=-SCALE)
                # vo[:, b, 0:64] = v_b * ek_b ; vo[:, b, 64] = ek_b
                for b in range(NB):
                    nc.vector.tensor_scalar_mul(
                        vo[:, b, 0:64], vn[:, b, :], ek[:, b : b + 1]
                    )
                nc.vector.tensor_copy(vo[:, :, 64:65], ek[:])

                if stage < 3:
                    ot1 = finp.tile([128, 65], f32, tag="ot1")
                    nc.vector.tensor_copy(ot1[:], vo[:, 0, :])
                    nc.sync.dma_start(out=o_p[h, 0:128, :], in_=ot1[:, 0:64])
                    continue
                # ---- transpose q,k -> [64, N]; fold 2*scale into qT --
                qT = kqtp.tile([64, n], QKD, tag="qT")
                kT = kqtp.tile([64, n], QKD, tag="kT")
                for b in range(NB):
                    ptq = ps_t.tile([128, 128], f32, tag="ptr")
                    nc.tensor.transpose(ptq[0:64, :], qn[:, b, :], ident[:])
                    nc.vector.tensor_scalar_mul(
                        qT[:, 128 * b : 128 * (b + 1)], ptq[0:64, :], 2.0 * SCALE
                    )
                    ptk = ps_t.tile([128, 128], f32, tag="ptr")
                    nc.tensor.transpose(ptk[0:64, :], kn[:, b, :], ident[:])
                    nc.vector.tensor_copy(
                        kT[:, 128 * b : 128 * (b + 1)], ptk[0:64, :]
                    )

                if stage < 4:
                    ot2 = finp.tile([64, 128], f32, tag="ot2")
                    nc.vector.tensor_copy(ot2[:], qT[:, 0:128])
                    nc.vector.tensor_mul(ot2[:], ot2[:], kT[:, 0:128])
                    nc.sync.dma_start(out=o_p[h, 0:64, :], in_=ot2[:, 0:64])
                    continue
                # ---- main flash loop ---------------------------------
                for t in range(NQT):
                    nblk = 4 * (t + 1)          # kv blocks for this q tile
                    ngrp = nblk // 2
                    acc = ps_a.tile([65, 512], f32, tag="acc")
                    qs = qT[:, 512 * t : 512 * (t + 1)]
                    for g in range(ngrp):
                        j0, j1 = 2 * g, 2 * g + 1
                        sp = ps_s.tile([128, 1024], f32, tag="sp")
                        nc.tensor.matmul(
                            sp[:, 0:512],
                            kT[:, 128 * j0 : 128 * (j0 + 1)],
                            qs,
                            start=True,
                            stop=True,
                        )
                        nc.tensor.matmul(
                            sp[:, 512:1024],
                            kT[:, 128 * j1 : 128 * (j1 + 1)],
                            qs,
                            start=True,
                            stop=True,
                        )
                        pt = ptp.tile([128, 1024], PVD, tag="pt")
                        nc.scalar.activation(pt[:], sp[:], Exp)
                        if stage < 5:
                            if g == 0:
                                nc.sync.dma_start(out=o_p[h, 128*t:128*(t+1), :].rearrange("p (a b) -> p a b", a=16), in_=pt[:].rearrange("p (a b) -> p a b", a=16))
                            continue
                        if g == 2 * t:        # diagonal pair r=(0,128)
                            nc.vector.tensor_mul(pt[:], pt[:], maskt[:, 0:1024])
                        elif g == 2 * t + 1:  # diagonal pair r=(256,384)
                            nc.vector.tensor_mul(pt[:], pt[:], maskt[:, 1024:2048])
                        nc.tensor.matmul(
                            acc[:],
                            vo[:, j0, :],
                            pt[:, 0:512],
                            start=(g == 0),
                            stop=False,
                        )
                        if debug_taps and h == 0 and t == 0 and g == 0:
                            nc.sync.dma_start(out=d_pt[:], in_=pt[:])
                        nc.tensor.matmul(
                            acc[:],
                            vo[:, j1, :],
                            pt[:, 512:1024],
                            start=False,
                            stop=(g == ngrp - 1),
                        )

                    # ---- finalize: transpose back, divide, store -----
                    acc_sb = finp.tile([65, 512], f32, tag="acc_sb")
                    nc.vector.tensor_copy(acc_sb[:], acc[:])
                    if debug_taps and h == 0 and t == 0:
                        nc.sync.dma_start(out=d_acc[:], in_=acc_sb[:])
                        nc.sync.dma_start(out=d_qT[:], in_=qT[:])
                        nc.sync.dma_start(out=d_kT[:], in_=kT[:])
                        nc.sync.dma_start(out=d_vo[:], in_=vo[:])
                    outt = finp.tile([128, 4, 64], f32, tag="outt")
                    dr = finp.tile([128, 8], f32, tag="dr")
                    for s in range(4):
                        ptr = ps_t.tile([128, 128], f32, tag="ptr")
                        nc.tensor.transpose(
                            ptr[:, 0:65],
                            acc_sb[:, 128 * s : 128 * (s + 1)],
                            ident[0:65, 0:65],
                        )
                        nc.vector.tensor_scalar_add(
                            dr[:, 2 * s : 2 * s + 1], ptr[:, 64:65], 1.0
                        )
                        nc.vector.reciprocal(
                            dr[:, 2 * s + 1 : 2 * s + 2], dr[:, 2 * s : 2 * s + 1]
                        )
                        nc.vector.tensor_scalar_mul(
                            outt[:, s, :], ptr[:, 0:64], dr[:, 2 * s + 1 : 2 * s + 2]
                        )
                    nc.sync.dma_start(
                        out=o_p[h].rearrange("(s p) d -> p s d", p=128)[
                            :, 4 * t : 4 * (t + 1), :
                        ],
                        in_=outt[:],
                    )

    nc.finalize()
    return nc


def _masks_np():
    j = np.arange(128)[:, None]
    c = np.arange(512)[None, :]
    cols = [(c - j >= r).astype(np.float32) for r in (0, 128, 256, 384)]
    return np.ascontiguousarray(np.concatenate(cols, axis=1))  # [128, 2048]


def get_program(qk_dt="float32r", pv_dt="float32r"):
    key = (qk_dt, pv_dt)
    if key not in _BUILT:
        _BUILT[key] = _build(qk_dt, pv_dt)
    return _BUILT[key]


def make_in_maps(q, k, v):
    """Split full [B,H,N,D] inputs into per-core input maps."""
    qf = np.asarray(q, dtype=np.float32).reshape(B * H, N, D)
    kf = np.asarray(k, dtype=np.float32).reshape(B * H, N, D)
    vf = np.asarray(v, dtype=np.float32).reshape(B * H, N, D)
    masks = _masks_np()
    maps = []
    for c in range(NCORES):
        sl = slice(c * HPC, (c + 1) * HPC)
        maps.append(
            {
                "q": np.ascontiguousarray(qf[sl]),
                "k": np.ascontiguousarray(kf[sl]),
                "v": np.ascontiguousarray(vf[sl]),
                "masks": masks,
            }
        )
    return maps


def kernel(q, k, v):
    from concourse.bass_utils import run_bass_kernel_spmd

    nc = get_program()
    maps = make_in_maps(q, k, v)
    res = run_bass_kernel_spmd(nc, maps, list(range(NCORES)))
    out = np.concatenate([res.results[c]["out"] for c in range(NCORES)], axis=0)
    return out.reshape(B, H, N, D)


# revision 15
# speedup vs baseline: 1.1834x; 1.1834x over previous
"""Trainium2 Bass kernel for nn_Attend (l2-distance attention with zero-kv).

Reference computation (per b,h):
    k' = [0; k], v' = [0; v]                       (prepend zero kv)
    scores[i,j] = (2 q_i.k'_j - |q_i|^2 - |k'_j|^2) * (D+2)^-0.5
    causal: j <= i+1 in padded index space
    out = softmax(scores) @ v'

Kernel algebra: softmax is invariant to the per-row constant -scale*|q_i|^2,
so with p~[i,j] = exp(2*scale*q_i.k_j) * exp(-scale*|k_j|^2) and the zero
column contributing exp(0)=1 to the denominator only:
    out_i = (sum_j p~ v_j) / (1 + sum_j p~)

Layout: scores are computed TRANSPOSED ([kv, q]) so P^T is directly the
moving operand of the PV matmul (no P transposes).  exp(-scale*|k|^2) is
folded into the PV stationary operand [V | 1] per kv partition; 2*scale is
folded into the exp activation's free affine scale.

q^T/k^T ([d, n] layout) are produced without touching the PE: a gpsimd
cast-DMA packs two heads fp32->bf16 into a [n, 128] DRAM staging buffer,
then one HWDGE DMA-transpose lands [128, n] in SBUF (head A = partitions
0:64, head B = 64:128).

Sharding: 32 (b,h) pairs -> 4 heads per core, 8 cores, pure data parallel.
"""

import sys

for _p in ("/opt/trn_rl_repo", "/root/.axon_site"):
    if _p not in sys.path:
        sys.path.insert(0, _p)

import numpy as np

B, H, N, D = 2, 16, 2048, 64
NCORES = 8
HPC = (B * H) // NCORES          # heads per core = 4
SCALE = float((D + 2) ** -0.5)   # augmented head dim, matches reference
NB = N // 128                    # kv blocks of 128 = 16
NQT = N // 512                   # q tiles of 512 = 4

_BUILT = {}


def _build(qk_dt="bfloat16", pv_dt="bfloat16", hpc=HPC, n=N, sgrp=2):
    """Build + finalize the SPMD Bass program (one core's view).

    sgrp: kv blocks per score-PSUM group (exp granularity), 2 or 3.
    """
    NB = n // 128
    NQT = n // 512
    import concourse.mybir as mybir
    import concourse.tile as tile
    from concourse import bacc
    from concourse.masks import make_identity

    f32 = mybir.dt.float32
    bf16 = mybir.dt.bfloat16
    QKD = getattr(mybir.dt, qk_dt)
    PVD = getattr(mybir.dt, pv_dt)
    Exp = mybir.ActivationFunctionType.Exp
    add = mybir.AluOpType.add

    nc = bacc.Bacc("TRN2", target_bir_lowering=False, debug=False)
    q_p = nc.declare_dram_parameter("q", [hpc, n, D], f32, isOutput=False)
    k_p = nc.declare_dram_parameter("k", [hpc, n, D], f32, isOutput=False)
    v_p = nc.declare_dram_parameter("v", [hpc, n, D], f32, isOutput=False)
    m_p = nc.declare_dram_parameter("masks", [128, 4 * 512], PVD, isOutput=False)
    o_p = nc.declare_dram_parameter("out", [hpc, n, D], f32, isOutput=True)

    npairs = (hpc + 1) // 2

    with tile.TileContext(nc) as tc:
        with (
            tc.tile_pool(name="stg", bufs=2, space="DRAM") as stgp,
            tc.tile_pool(name="const", bufs=1) as constp,
            tc.tile_pool(name="io", bufs=2) as iop,
            tc.tile_pool(name="kqt", bufs=2) as kqtp,
            tc.tile_pool(name="pt", bufs=3) as ptp,
            tc.tile_pool(name="fin", bufs=2) as finp,
            tc.tile_pool(name="ps_s", bufs=2, space="PSUM") as ps_s,
            tc.tile_pool(name="ps_a", bufs=2, space="PSUM") as ps_a,
            tc.tile_pool(name="ps_t", bufs=2, space="PSUM") as ps_t,
        ):
            ident = constp.tile([128, 128], f32, tag="ident")
            make_identity(nc, ident[:])
            maskt = constp.tile([128, 4 * 512], PVD, tag="maskt")
            nc.sync.dma_start(out=maskt[:], in_=m_p[:])

            for pair in range(npairs):
                h0 = 2 * pair
                ph = [h for h in (h0, h0 + 1) if h < hpc]
                two_byte = mybir.dt.size(QKD) == 2
                if two_byte:
                    # ---- qT/kT via cast-DMA + DMA-transpose ----------
                    stq = stgp.tile([n, 128], QKD, tag="stq")
                    stk = stgp.tile([n, 128], QKD, tag="stk")
                    for i, h in enumerate(ph):
                        nc.gpsimd.dma_start(
                            out=stq[:, 64 * i : 64 * i + 64], in_=q_p[h]
                        )
                        nc.gpsimd.dma_start(
                            out=stk[:, 64 * i : 64 * i + 64], in_=k_p[h]
                        )
                    qT2 = kqtp.tile([128, n], QKD, tag="qT2")
                    kT2 = kqtp.tile([128, n], QKD, tag="kT2")
                    nc.sync.dma_start(out=qT2[:], in_=stq[:], transpose=True)
                    nc.sync.dma_start(out=kT2[:], in_=stk[:], transpose=True)

                for i, h in enumerate(ph):
                    if two_byte:
                        qT = qT2[64 * i : 64 * i + 64, :]
                        kT = kT2[64 * i : 64 * i + 64, :]
                    else:
                        # ---- PE-transpose fallback (4-byte dtypes) ---
                        qn = iop.tile([128, NB, 64], f32, tag="qn")
                        nc.sync.dma_start(
                            out=qn[:],
                            in_=q_p[h].rearrange("(b p) d -> p b d", p=128),
                        )
                        kn0 = iop.tile([128, NB, 64], f32, tag="kn0")
                        nc.sync.dma_start(
                            out=kn0[:],
                            in_=k_p[h].rearrange("(b p) d -> p b d", p=128),
                        )
                        qTt = kqtp.tile([64, n], QKD, tag="qTt")
                        kTt = kqtp.tile([64, n], QKD, tag="kTt")
                        for b in range(NB):
                            ptq = ps_t.tile([128, 128], f32, tag="ptr")
                            nc.tensor.transpose(
                                ptq[0:64, :], qn[:, b, :], ident[:]
                            )
                            nc.vector.tensor_copy(
                                qTt[:, 128 * b : 128 * (b + 1)], ptq[0:64, :]
                            )
                            ptk = ps_t.tile([128, 128], f32, tag="ptr")
                            nc.tensor.transpose(
                                ptk[0:64, :], kn0[:, b, :], ident[:]
                            )
                            nc.vector.tensor_copy(
                                kTt[:, 128 * b : 128 * (b + 1)], ptk[0:64, :]
                            )
                        qT = qTt[:]
                        kT = kTt[:]
                    # ---- load head k (fp32 for |k|^2) and v ----------
                    kn = iop.tile([128, NB, 64], f32, tag="kn")
                    vn = iop.tile([128, NB, 64], f32, tag="vn")
                    vo = iop.tile([128, NB, 65], PVD, tag="vo")
                    nc.sync.dma_start(
                        out=kn[:], in_=k_p[h].rearrange("(b p) d -> p b d", p=128)
                    )
                    nc.sync.dma_start(
                        out=vn[:], in_=v_p[h].rearrange("(b p) d -> p b d", p=128)
                    )

                    # ---- ek = exp(-scale*|k|^2) folded into [V|1] ----
                    ksqs = iop.tile([128, NB], f32, tag="ksqs")
                    scr = iop.tile([128, 64], f32, tag="scr")
                    for b in range(NB):
                        nc.vector.tensor_mul(scr[:], kn[:, b, :], kn[:, b, :])
                        nc.vector.tensor_reduce(
                            ksqs[:, b : b + 1], scr[:], mybir.AxisListType.X, add
                        )
                    ek = iop.tile([128, NB], f32, tag="ek")
                    nc.scalar.activation(ek[:], ksqs[:], Exp, scale=-SCALE)
                    for b in range(NB):
                        nc.vector.tensor_scalar_mul(
                            vo[:, b, 0:64], vn[:, b, :], ek[:, b : b + 1]
                        )
                    nc.vector.tensor_copy(vo[:, :, 64:65], ek[:])

                    # ---- main flash loop -----------------------------
                    for t in range(NQT):
                        nblk = 4 * (t + 1)
                        ngrp = nblk // 2
                        acc = ps_a.tile([65, 512], f32, tag="acc")
                        qs = qT[:, 512 * t : 512 * (t + 1)]
                        for g in range(ngrp):
                            j0, j1 = 2 * g, 2 * g + 1
                            sp = ps_s.tile([128, 1024], f32, tag="sp")
                            nc.tensor.matmul(
                                sp[:, 0:512],
                                kT[:, 128 * j0 : 128 * (j0 + 1)],
                                qs,
                                start=True,
                                stop=True,
                            )
                            nc.tensor.matmul(
                                sp[:, 512:1024],
                                kT[:, 128 * j1 : 128 * (j1 + 1)],
                                qs,
                                start=True,
                                stop=True,
                            )
                            pt = ptp.tile([128, 1024], PVD, tag="pt")
                            nc.scalar.activation(pt[:], sp[:], Exp, scale=2.0 * SCALE)
                            if g == 2 * t:        # diagonal pair r=(0,128)
                                nc.vector.tensor_mul(
                                    pt[:], pt[:], maskt[:, 0:1024]
                                )
                            elif g == 2 * t + 1:  # diagonal pair r=(256,384)
                                nc.vector.tensor_mul(
                                    pt[:], pt[:], maskt[:, 1024:2048]
                                )
                            nc.tensor.matmul(
                                acc[:],
                                vo[:, j0, :],
                                pt[:, 0:512],
                                start=(g == 0),
                                stop=False,
                            )
                            nc.tensor.matmul(
                                acc[:],
                                vo[:, j1, :],
                                pt[:, 512:1024],
                                start=False,
                                stop=(g == ngrp - 1),
                            )

                        # ---- finalize: transpose, divide, store ------
                        acc_sb = finp.tile([65, 512], f32, tag="acc_sb")
                        nc.vector.tensor_copy(acc_sb[:], acc[:])
                        outt = finp.tile([128, 4, 64], f32, tag="outt")
                        dr = finp.tile([128, 8], f32, tag="dr")
                        for s in range(4):
                            ptr = ps_t.tile([128, 128], f32, tag="ptr")
                            nc.tensor.transpose(
                                ptr[:, 0:65],
                                acc_sb[:, 128 * s : 128 * (s + 1)],
                                ident[0:65, 0:65],
                            )
                            nc.vector.tensor_scalar_add(
                                dr[:, 2 * s : 2 * s + 1], ptr[:, 64:65], 1.0
                            )
                            nc.vector.reciprocal(
                                dr[:, 2 * s + 1 : 2 * s + 2],
                                dr[:, 2 * s : 2 * s + 1],
                            )
                            nc.vector.tensor_scalar_mul(
                                outt[:, s, :],
                                ptr[:, 0:64],
                                dr[:, 2 * s + 1 : 2 * s + 2],
                            )
                        nc.sync.dma_start(
                            out=o_p[h].rearrange("(s p) d -> p s d", p=128)[
                                :, 4 * t : 4 * (t + 1), :
                            ],
                            in_=outt[:],
                        )

    nc.finalize()
    return nc


def _masks_np(dtype_name="bfloat16"):
    import ml_dtypes

    dt = np.float32 if dtype_name.startswith("float32") else ml_dtypes.bfloat16
    j = np.arange(128)[:, None]
    c = np.arange(512)[None, :]
    cols = [(c - j >= r).astype(dt) for r in (0, 128, 256, 384)]
    return np.ascontiguousarray(np.concatenate(cols, axis=1))  # [128, 2048]


def get_program(qk_dt="bfloat16", pv_dt="bfloat16"):
    key = (qk_dt, pv_dt)
    if key not in _BUILT:
        _BUILT[key] = _build(qk_dt, pv_dt)
    return _BUILT[key]


def make_in_maps(q, k, v, pv_dt="bfloat16"):
    """Split full [B,H,N,D] inputs into per-core input maps."""
    qf = np.asarray(q, dtype=np.float32).reshape(B * H, N, D)
    kf = np.asarray(k, dtype=np.float32).reshape(B * H, N, D)
    vf = np.asarray(v, dtype=np.float32).reshape(B * H, N, D)
    masks = _masks_np(pv_dt)
    maps = []
    for c in range(NCORES):
        sl = slice(c * HPC, (c + 1) * HPC)
        maps.append(
            {
                "q": np.ascontiguousarray(qf[sl]),
                "k": np.ascontiguousarray(kf[sl]),
                "v": np.ascontiguousarray(vf[sl]),
                "masks": masks,
            }
        )
    return maps


def kernel(q, k, v):
    from concourse.bass_utils import run_bass_kernel_spmd

    nc = get_program()
    maps = make_in_maps(q, k, v)
    res = run_bass_kernel_spmd(nc, maps, list(range(NCORES)))
    out = np.concatenate([res.results[c]["out"] for c in range(NCORES)], axis=0)
    return out.reshape(B, H, N, D)


# revision 17
# speedup vs baseline: 1.4517x; 1.2267x over previous
"""Trainium2 Bass kernel for nn_Attend (l2-distance attention with zero-kv).

Reference computation (per b,h):
    k' = [0; k], v' = [0; v]                       (prepend zero kv)
    scores[i,j] = (2 q_i.k'_j - |q_i|^2 - |k'_j|^2) * (D+2)^-0.5
    causal: j <= i+1 in padded index space
    out = softmax(scores) @ v'

Kernel algebra: softmax is invariant to the per-row constant -scale*|q_i|^2,
so with p~[i,j] = exp(2*scale*q_i.k_j) * exp(-scale*|k_j|^2) and the zero
column contributing exp(0)=1 to the denominator only:
    out_i = (sum_j p~ v_j) / (1 + sum_j p~)

Layout: scores are computed TRANSPOSED ([kv, q]) so P^T is directly the
moving operand of the PV matmul (no P transposes).  exp(-scale*|k|^2) is
folded into the PV stationary operand [V | 1] per kv partition; 2*scale is
folded into the exp activation's free affine scale.

The PE streams the moving operand at half rate when the contraction dim is
<= 64, so heads are processed in PAIRS with K=128: kT2 [128, n] stacks both
heads' k^T; q^T is staged BLOCK-DIAGONALLY (qTp [128, 2n]: head A in rows
0:64 of the first n cols, head B in rows 64:128 of the last n cols, zeros
elsewhere) so one K=128 matmul per head yields that head's scores with the
other head's contribution zeroed.  q^T/k^T are produced without the PE:
gpsimd cast-DMA (fp32->bf16) into DRAM staging, then HWDGE DMA-transpose.

Sharding: 32 (b,h) pairs -> 4 heads per core, 8 cores, pure data parallel.
"""

import sys

for _p in ("/opt/trn_rl_repo", "/root/.axon_site"):
    if _p not in sys.path:
        sys.path.insert(0, _p)

import numpy as np

B, H, N, D = 2, 16, 2048, 64
NCORES = 8
HPC = (B * H) // NCORES          # heads per core = 4
SCALE = float((D + 2) ** -0.5)   # augmented head dim, matches reference
NB = N // 128                    # kv blocks of 128 = 16
NQT = N // 512                   # q tiles of 512 = 4

_BUILT = {}


def _build(qk_dt="bfloat16", pv_dt="bfloat16", hpc=HPC, n=N):
    """Build + finalize the SPMD Bass program (one core's view)."""
    assert qk_dt == "bfloat16" and pv_dt == "bfloat16", "v3 builder is bf16-only"
    assert hpc % 2 == 0, "heads processed in pairs"
    NB = n // 128
    NQT = n // 512
    import concourse.mybir as mybir
    import concourse.tile as tile
    from concourse import bacc
    from concourse.masks import make_identity

    f32 = mybir.dt.float32
    bf16 = mybir.dt.bfloat16
    Exp = mybir.ActivationFunctionType.Exp
    add = mybir.AluOpType.add

    nc = bacc.Bacc("TRN2", target_bir_lowering=False, debug=False)
    q_p = nc.declare_dram_parameter("q", [hpc, n, D], f32, isOutput=False)
    k_p = nc.declare_dram_parameter("k", [hpc, n, D], f32, isOutput=False)
    v_p = nc.declare_dram_parameter("v", [hpc, n, D], f32, isOutput=False)
    m_p = nc.declare_dram_parameter("masks", [128, 4 * 1024], bf16, isOutput=False)
    o_p = nc.declare_dram_parameter("out", [hpc, n, D], f32, isOutput=True)

    npairs = hpc // 2

    with tile.TileContext(nc) as tc:
        with (
            tc.tile_pool(name="stg", bufs=2, space="DRAM") as stgp,
            tc.tile_pool(name="const", bufs=1) as constp,
            tc.tile_pool(name="io", bufs=2) as iop,
            tc.tile_pool(name="kqt", bufs=2) as kqtp,
            tc.tile_pool(name="pt", bufs=4) as ptp,
            tc.tile_pool(name="fin", bufs=2) as finp,
            tc.tile_pool(name="ps_s", bufs=2, space="PSUM") as ps_s,
            tc.tile_pool(name="ps_af", bufs=4, space="PSUM") as ps_af,
        ):
            ident = constp.tile([128, 128], f32, tag="ident")
            make_identity(nc, ident[:])
            maskt = constp.tile([128, 4 * 1024], bf16, tag="maskt")
            nc.sync.dma_start(out=maskt[:], in_=m_p[:])
            zt = constp.tile([128, 2 * n], bf16, tag="zt")
            nc.vector.memset(zt[:], 0.0)  # [128, 2n] bf16 = 2n*128 elems

            for pair in range(npairs):
                hA, hB = 2 * pair, 2 * pair + 1
                # ---- staging: zero, cast-pack, DMA-transpose ---------
                stq = stgp.tile([2 * n, 128], bf16, tag="stq")
                stk = stgp.tile([n, 128], bf16, tag="stk")
                nc.sync.dma_start(out=stq[:], in_=zt[:])
                nc.gpsimd.dma_start(out=stq[0:n, 0:64], in_=q_p[hA])
                nc.gpsimd.dma_start(out=stq[n : 2 * n, 64:128], in_=q_p[hB])
                nc.gpsimd.dma_start(out=stk[:, 0:64], in_=k_p[hA])
                nc.gpsimd.dma_start(out=stk[:, 64:128], in_=k_p[hB])
                qTp = kqtp.tile([128, 2 * n], bf16, tag="qTp")
                kT2 = kqtp.tile([128, n], bf16, tag="kT2")
                nc.sync.dma_start(out=qTp[:], in_=stq[:], transpose=True)
                nc.sync.dma_start(out=kT2[:], in_=stk[:], transpose=True)

                # ---- per-head: [V*ek | ek] --------------------------
                vos = []
                for h in (hA, hB):
                    kn = iop.tile([128, NB, 64], f32, tag="kn")
                    vn = iop.tile([128, NB, 64], f32, tag="vn")
                    vo = iop.tile([128, NB, 65], bf16, tag="vo")
                    nc.sync.dma_start(
                        out=kn[:], in_=k_p[h].rearrange("(b p) d -> p b d", p=128)
                    )
                    nc.sync.dma_start(
                        out=vn[:], in_=v_p[h].rearrange("(b p) d -> p b d", p=128)
                    )
                    scr2 = iop.tile([128, NB, 64], f32, tag="scr2")
                    ksqs = iop.tile([128, NB], f32, tag="ksqs")
                    nc.vector.tensor_mul(scr2[:], kn[:], kn[:])
                    nc.vector.tensor_reduce(
                        ksqs[:], scr2[:], mybir.AxisListType.X, add
                    )
                    ek = iop.tile([128, NB], f32, tag="ek")
                    nc.scalar.activation(ek[:], ksqs[:], Exp, scale=-SCALE)
                    for b in range(NB):
                        nc.vector.tensor_scalar_mul(
                            vo[:, b, 0:64], vn[:, b, :], ek[:, b : b + 1]
                        )
                    nc.vector.tensor_copy(vo[:, :, 64:65], ek[:])
                    vos.append(vo)
                voA, voB = vos

                # ---- main flash loop (both heads per block) ----------
                for t in range(NQT):
                    nblk = 4 * (t + 1)
                    accA = ps_af.tile([65, 512], f32, tag="af", name=f"accA_{pair}_{t}")
                    accB = ps_af.tile([65, 512], f32, tag="af", name=f"accB_{pair}_{t}")
                    qsA = qTp[:, 512 * t : 512 * (t + 1)]
                    qsB = qTp[:, n + 512 * t : n + 512 * (t + 1)]
                    for j in range(nblk):
                        kslc = kT2[:, 128 * j : 128 * (j + 1)]
                        sp = ps_s.tile([128, 1024], f32, tag="sp")
                        nc.tensor.matmul(
                            sp[:, 0:512], kslc, qsA, start=True, stop=True
                        )
                        nc.tensor.matmul(
                            sp[:, 512:1024], kslc, qsB, start=True, stop=True
                        )
                        pt = ptp.tile([128, 1024], bf16, tag="pt")
                        nc.scalar.activation(pt[:], sp[:], Exp, scale=2.0 * SCALE)
                        r = j - 4 * t
                        if 0 <= r < 4:  # diagonal block: mask both halves
                            nc.vector.tensor_mul(
                                pt[:], pt[:], maskt[:, 1024 * r : 1024 * (r + 1)]
                            )
                        nc.tensor.matmul(
                            accA[:],
                            voA[:, j, :],
                            pt[:, 0:512],
                            start=(j == 0),
                            stop=(j == nblk - 1),
                        )
                        nc.tensor.matmul(
                            accB[:],
                            voB[:, j, :],
                            pt[:, 512:1024],
                            start=(j == 0),
                            stop=(j == nblk - 1),
                        )

                    # ---- finalize both heads -------------------------
                    for h, acc in ((hA, accA), (hB, accB)):
                        acc_sb = finp.tile([65, 512], f32, tag="acc_sb")
                        nc.vector.tensor_copy(acc_sb[:], acc[:])
                        ptr4 = ps_af.tile(
                            [128, 4, 65], f32, tag="af", name=f"ptr4_{pair}_{t}_{h}"
                        )
                        for s in range(4):
                            nc.tensor.matmul(
                                ptr4[:, s, :],
                                acc_sb[:, 128 * s : 128 * (s + 1)],
                                ident[0:65, 0:65],
                                is_transpose=True,
                                start=(s == 0),
                                stop=(s == 3),
                            )
                        outt = finp.tile([128, 4, 64], f32, tag="outt")
                        dr = finp.tile([128, 8], f32, tag="dr")
                        nc.vector.tensor_scalar_add(
                            dr[:, 0:4], ptr4[:, :, 64], 1.0
                        )
                        nc.vector.reciprocal(dr[:, 4:8], dr[:, 0:4])
                        for s in range(4):
                            nc.vector.tensor_scalar_mul(
                                outt[:, s, :],
                                ptr4[:, s, 0:64],
                                dr[:, 4 + s : 5 + s],
                            )
                        nc.sync.dma_start(
                            out=o_p[h].rearrange("(s p) d -> p s d", p=128)[
                                :, 4 * t : 4 * (t + 1), :
                            ],
                            in_=outt[:],
                        )

    nc.finalize()
    return nc


def _masks_np(dtype_name="bfloat16"):
    import ml_dtypes

    dt = np.float32 if dtype_name.startswith("float32") else ml_dtypes.bfloat16
    j = np.arange(128)[:, None]
    c = np.arange(512)[None, :]
    cols = []
    for r in (0, 128, 256, 384):
        m = (c - j >= r).astype(dt)
        cols.append(m)
        cols.append(m)  # duplicated for the two heads of a pair
    return np.ascontiguousarray(np.concatenate(cols, axis=1))  # [128, 4096]


def get_program(qk_dt="bfloat16", pv_dt="bfloat16"):
    key = (qk_dt, pv_dt)
    if key not in _BUILT:
        _BUILT[key] = _build(qk_dt, pv_dt)
    return _BUILT[key]


def make_in_maps(q, k, v, pv_dt="bfloat16"):
    """Split full [B,H,N,D] inputs into per-core input maps."""
    qf = np.asarray(q, dtype=np.float32).reshape(B * H, N, D)
    kf = np.asarray(k, dtype=np.float32).reshape(B * H, N, D)
    vf = np.asarray(v, dtype=np.float32).reshape(B * H, N, D)
    masks = _masks_np(pv_dt)
    maps = []
    for c in range(NCORES):
        sl = slice(c * HPC, (c + 1) * HPC)
        maps.append(
            {
                "q": np.ascontiguousarray(qf[sl]),
                "k": np.ascontiguousarray(kf[sl]),
                "v": np.ascontiguousarray(vf[sl]),
                "masks": masks,
            }
        )
    return maps


def kernel(q, k, v):
    from concourse.bass_utils import run_bass_kernel_spmd

    nc = get_program()
    maps = make_in_maps(q, k, v)
    res = run_bass_kernel_spmd(nc, maps, list(range(NCORES)))
    out = np.concatenate([res.results[c]["out"] for c in range(NCORES)], axis=0)
    return out.reshape(B, H, N, D)


# revision 18
# speedup vs baseline: 1.4777x; 1.0179x over previous
"""Trainium2 Bass kernel for nn_Attend (l2-distance attention with zero-kv).

Reference computation (per b,h):
    k' = [0; k], v' = [0; v]                       (prepend zero kv)
    scores[i,j] = (2 q_i.k'_j - |q_i|^2 - |k'_j|^2) * (D+2)^-0.5
    causal: j <= i+1 in padded index space
    out = softmax(scores) @ v'

Kernel algebra: softmax is invariant to the per-row constant -scale*|q_i|^2,
so with p~[i,j] = exp(2*scale*q_i.k_j) * exp(-scale*|k_j|^2) and the zero
column contributing exp(0)=1 to the denominator only:
    out_i = (sum_j p~ v_j) / (1 + sum_j p~)

Layout: scores are computed TRANSPOSED ([kv, q]) so P^T is directly the
moving operand of the PV matmul (no P transposes).  exp(-scale*|k|^2) is
folded into the PV stationary operand [V | 1] per kv partition; 2*scale is
folded into the exp activation's free affine scale.

The PE streams the moving operand at half rate when the contraction dim is
<= 64, so heads are processed in PAIRS with K=128: kT2 [128, n] stacks both
heads' k^T; q^T is staged BLOCK-DIAGONALLY (qTp [128, 2n]: head A in rows
0:64 of the first n cols, head B in rows 64:128 of the last n cols, zeros
elsewhere) so one K=128 matmul per head yields that head's scores with the
other head's contribution zeroed.  q^T/k^T are produced without the PE:
gpsimd cast-DMA (fp32->bf16) into DRAM staging, then HWDGE DMA-transpose.

Sharding: 32 (b,h) pairs -> 4 heads per core, 8 cores, pure data parallel.
"""

import sys

for _p in ("/opt/trn_rl_repo", "/root/.axon_site"):
    if _p not in sys.path:
        sys.path.insert(0, _p)

import numpy as np

B, H, N, D = 2, 16, 2048, 64
NCORES = 8
HPC = (B * H) // NCORES          # heads per core = 4
SCALE = float((D + 2) ** -0.5)   # augmented head dim, matches reference
NB = N // 128                    # kv blocks of 128 = 16
NQT = N // 512                   # q tiles of 512 = 4

_BUILT = {}


def _build(qk_dt="bfloat16", pv_dt="bfloat16", hpc=HPC, n=N):
    """Build + finalize the SPMD Bass program (one core's view)."""
    assert qk_dt == "bfloat16" and pv_dt == "bfloat16", "v3 builder is bf16-only"
    assert hpc % 2 == 0, "heads processed in pairs"
    NB = n // 128
    NQT = n // 512
    import concourse.mybir as mybir
    import concourse.tile as tile
    from concourse import bacc
    from concourse.masks import make_identity

    f32 = mybir.dt.float32
    bf16 = mybir.dt.bfloat16
    Exp = mybir.ActivationFunctionType.Exp
    add = mybir.AluOpType.add

    nc = bacc.Bacc("TRN2", target_bir_lowering=False, debug=False, num_swdge_queues=4)
    q_p = nc.declare_dram_parameter("q", [hpc, n, D], f32, isOutput=False)
    k_p = nc.declare_dram_parameter("k", [hpc, n, D], f32, isOutput=False)
    v_p = nc.declare_dram_parameter("v", [hpc, n, D], f32, isOutput=False)
    m_p = nc.declare_dram_parameter("masks", [128, 4 * 1024], bf16, isOutput=False)
    o_p = nc.declare_dram_parameter("out", [hpc, n, D], f32, isOutput=True)

    npairs = hpc // 2

    with tile.TileContext(nc) as tc:
        with (
            tc.tile_pool(name="stg", bufs=2, space="DRAM") as stgp,
            tc.tile_pool(name="const", bufs=1) as constp,
            tc.tile_pool(name="io", bufs=2) as iop,
            tc.tile_pool(name="kqt", bufs=2) as kqtp,
            tc.tile_pool(name="pt", bufs=4) as ptp,
            tc.tile_pool(name="fin", bufs=2) as finp,
            tc.tile_pool(name="vop", bufs=4) as vop,
            tc.tile_pool(name="ps_s", bufs=3, space="PSUM") as ps_s,
            tc.tile_pool(name="ps_af", bufs=2, space="PSUM") as ps_af,
        ):
            ident = constp.tile([128, 128], f32, tag="ident")
            make_identity(nc, ident[:])
            maskt = constp.tile([128, 4 * 1024], bf16, tag="maskt")
            nc.scalar.dma_start(out=maskt[:], in_=m_p[:])

            # ---- staging for ALL pairs up-front ----------------------
            qTps, kT2s = [], []
            for pair in range(npairs):
                hA, hB = 2 * pair, 2 * pair + 1
                stq = stgp.tile([n, 128], bf16, tag="stq")
                stk = stgp.tile([n, 128], bf16, tag="stk")
                nc.gpsimd.dma_start(out=stq[:, 0:64], in_=q_p[hA])
                nc.gpsimd.dma_start(out=stq[:, 64:128], in_=q_p[hB])
                nc.gpsimd.dma_start(out=stk[:, 0:64], in_=k_p[hA])
                nc.gpsimd.dma_start(out=stk[:, 64:128], in_=k_p[hB])
                qT2 = kqtp.tile([128, n], bf16, tag="qT2", name=f"qT2_{pair}")
                kT2 = kqtp.tile([128, n], bf16, tag="kT2", name=f"kT2_{pair}")
                nc.sync.dma_start(out=qT2[:], in_=stq[:], transpose=True)
                nc.sync.dma_start(out=kT2[:], in_=stk[:], transpose=True)
                # block-diagonal qTp assembled on-chip
                qTp = kqtp.tile([128, 2 * n], bf16, tag="qTp", name=f"qTp_{pair}")
                nc.vector.tensor_copy(qTp[0:64, 0:n], qT2[0:64, :])
                nc.vector.memset(qTp[64:128, 0:n], 0.0)
                nc.vector.memset(qTp[0:64, n : 2 * n], 0.0)
                nc.vector.tensor_copy(qTp[64:128, n : 2 * n], qT2[64:128, :])
                qTps.append(qTp)
                kT2s.append(kT2)

            for pair in range(npairs):
                hA, hB = 2 * pair, 2 * pair + 1
                qTp = qTps[pair]
                kT2 = kT2s[pair]

                # ---- per-head: [V*ek | ek] --------------------------
                vos = []
                for h in (hA, hB):
                    kn = iop.tile([128, NB, 64], f32, tag="kn")
                    vn = iop.tile([128, NB, 64], f32, tag="vn")
                    vo = vop.tile([128, NB, 65], bf16, tag="vo")
                    nc.scalar.dma_start(
                        out=kn[:], in_=k_p[h].rearrange("(b p) d -> p b d", p=128)
                    )
                    nc.scalar.dma_start(
                        out=vn[:], in_=v_p[h].rearrange("(b p) d -> p b d", p=128)
                    )
                    scr2 = iop.tile([128, NB, 64], f32, tag="scr2")
                    ksqs = iop.tile([128, NB], f32, tag="ksqs")
                    nc.vector.tensor_mul(scr2[:], kn[:], kn[:])
                    nc.vector.tensor_reduce(
                        ksqs[:], scr2[:], mybir.AxisListType.X, add
                    )
                    ek = iop.tile([128, NB], f32, tag="ek")
                    nc.scalar.activation(ek[:], ksqs[:], Exp, scale=-SCALE)
                    for b in range(NB):
                        nc.vector.tensor_scalar_mul(
                            vo[:, b, 0:64], vn[:, b, :], ek[:, b : b + 1]
                        )
                    nc.vector.tensor_copy(vo[:, :, 64:65], ek[:])
                    vos.append(vo)
                voA, voB = vos

                # ---- main flash loop (both heads per block) ----------
                for t in range(NQT):
                    nblk = 4 * (t + 1)
                    accA = ps_af.tile([65, 512], f32, tag="af", name=f"accA_{pair}_{t}")
                    accB = ps_af.tile([65, 512], f32, tag="af", name=f"accB_{pair}_{t}")
                    qsA = qTp[:, 512 * t : 512 * (t + 1)]
                    qsB = qTp[:, n + 512 * t : n + 512 * (t + 1)]
                    for j in range(nblk):
                        kslc = kT2[:, 128 * j : 128 * (j + 1)]
                        sp = ps_s.tile([128, 1024], f32, tag="sp")
                        nc.tensor.matmul(
                            sp[:, 0:512], kslc, qsA, start=True, stop=True
                        )
                        nc.tensor.matmul(
                            sp[:, 512:1024], kslc, qsB, start=True, stop=True
                        )
                        pt = ptp.tile([128, 1024], bf16, tag="pt")
                        nc.scalar.activation(pt[:], sp[:], Exp, scale=2.0 * SCALE)
                        r = j - 4 * t
                        if 0 <= r < 4:  # diagonal block: mask both halves
                            nc.vector.tensor_mul(
                                pt[:], pt[:], maskt[:, 1024 * r : 1024 * (r + 1)]
                            )
                        nc.tensor.matmul(
                            accA[:],
                            voA[:, j, :],
                            pt[:, 0:512],
                            start=(j == 0),
                            stop=(j == nblk - 1),
                        )
                        nc.tensor.matmul(
                            accB[:],
                            voB[:, j, :],
                            pt[:, 512:1024],
                            start=(j == 0),
                            stop=(j == nblk - 1),
                        )

                    # ---- finalize both heads -------------------------
                    for h, acc in ((hA, accA), (hB, accB)):
                        acc_sb = finp.tile([65, 512], f32, tag="acc_sb")
                        nc.vector.tensor_copy(acc_sb[:], acc[:])
                        ptr4 = ps_s.tile(
                            [128, 4, 65], f32, tag="sp", name=f"ptr4_{pair}_{t}_{h}"
                        )
                        for s in range(4):
                            nc.tensor.matmul(
                                ptr4[:, s, :],
                                acc_sb[:, 128 * s : 128 * (s + 1)],
                                ident[0:65, 0:65],
                                is_transpose=True,
                                start=(s == 0),
                                stop=(s == 3),
                            )
                        outt = finp.tile([128, 4, 64], f32, tag="outt")
                        dr = finp.tile([128, 8], f32, tag="dr")
                        nc.vector.tensor_scalar_add(
                            dr[:, 0:4], ptr4[:, :, 64], 1.0
                        )
                        nc.vector.reciprocal(dr[:, 4:8], dr[:, 0:4])
                        for s in range(4):
                            nc.vector.tensor_scalar_mul(
                                outt[:, s, :],
                                ptr4[:, s, 0:64],
                                dr[:, 4 + s : 5 + s],
                            )
                        nc.scalar.dma_start(
                            out=o_p[h].rearrange("(s p) d -> p s d", p=128)[
                                :, 4 * t : 4 * (t + 1), :
                            ],
                            in_=outt[:],
                        )

    nc.finalize()
    return nc


def _masks_np(dtype_name="bfloat16"):
    import ml_dtypes

    dt = np.float32 if dtype_name.startswith("float32") else ml_dtypes.bfloat16
    j = np.arange(128)[:, None]
    c = np.arange(512)[None, :]
    cols = []
    for r in (0, 128, 256, 384):
        m = (c - j >= r).astype(dt)
        cols.append(m)
        cols.append(m)  # duplicated for the two heads of a pair
    return np.ascontiguousarray(np.concatenate(cols, axis=1))  # [128, 4096]


def get_program(qk_dt="bfloat16", pv_dt="bfloat16"):
    key = (qk_dt, pv_dt)
    if key not in _BUILT:
        _BUILT[key] = _build(qk_dt, pv_dt)
    return _BUILT[key]


def make_in_maps(q, k, v, pv_dt="bfloat16"):
    """Split full [B,H,N,D] inputs into per-core input maps."""
    qf = np.asarray(q, dtype=np.float32).reshape(B * H, N, D)
    kf = np.asarray(k, dtype=np.float32).reshape(B * H, N, D)
    vf = np.asarray(v, dtype=np.float32).reshape(B * H, N, D)
    masks = _masks_np(pv_dt)
    maps = []
    for c in range(NCORES):
        sl = slice(c * HPC, (c + 1) * HPC)
        maps.append(
            {
                "q": np.ascontiguousarray(qf[sl]),
                "k": np.ascontiguousarray(kf[sl]),
                "v": np.ascontiguousarray(vf[sl]),
                "masks": masks,
            }
        )
    return maps


def kernel(q, k, v):
    from concourse.bass_utils import run_bass_kernel_spmd

    nc = get_program()
    maps = make_in_maps(q, k, v)
    res = run_bass_kernel_spmd(nc, maps, list(range(NCORES)))
    out = np.concatenate([res.results[c]["out"] for c in range(NCORES)], axis=0)
    return out.reshape(B, H, N, D)
